# revision 36
# baseline (speedup 1.0000x reference)
"""Cosformer attention (causal linear attention with cos reweighting) on 8
Trainium2 NeuronCores.

Sharding: n = bsz*heads = 16 sequences -> 2 per core. Core c handles batch-half
i = c//4 and head-pair p = c%4 (heads 2p, 2p+1). Fully data/head parallel; the
only cross-core interaction is the host-side sum of output-projection partials.

v5: DMA-queue-aware layout. Per-queue DMA bandwidth is ~40-95 GB/s (not the
358 GB/s aggregate), so loads/stores are packed into few wide-row transfers
and spread across all three queues (sync HWDGE, scalar HWDGE, gpsimd SWDGE):
 - xt is packed [e0_th0|e1_th0|e0_th1|e1_th1] so each token-half is ONE dma.
 - output partials are written as chunk PAIRS ([128,1024] -> 256 dram rows).
 - k^T comes from XBAR dma_start_transpose (one per head per half), attn^T
   from per-chunk PE transposes (keeps the out-proj pipeline short).
 - persistent vt pair tiles with preset ones-columns; one strided pair copy
   per chunk instead of per-head copies + memsets.
 - qkv and the running state are head-PAIR PSUM tiles (one bank each; head
   a's start=True zeroes the whole bank, head b rides with start=False).
 - 8 warmup matmuls on p1 ramp the PE pstate during the load stream.

Per-core kernel (L=1024 tokens, d=64 per head, pair feature dim P=128):
  1. Feat-major projections per head (duplicated-W trick) -> relu(+bias) ->
     * [sin;cos] row table -> bf16 q_^T,k_^T. V^T projected once per pair.
  2. Chunked causal linear attention (bf16 matmuls, fp32 PSUM), chunk=128:
       B    = masked A^T (upper-tri j<=i)
       qkv  = B.T @ V~  +  q^T.T @ S     (V~ = [V|1]; col 64 = denominator)
       S   += K_tok.T @ V~ in a persistent PSUM bank (fp32, no drift)
       attn = qkv[:,0:64] * 1/max(denom,eps)  -> attn_all (token-major)
  3. per-chunk PE transpose -> bf16 out-proj partial -> paired DRAM stores.
Host sums 4 partials per batch-half in f32, adds bo, reinterleaves rows.
"""

import os
import sys

import numpy as np

for _p in ("/opt/trn_rl_repo", "/root/.axon_site/_ro/trn_rl_repo"):
    if os.path.isdir(_p) and _p not in sys.path:
        sys.path.insert(0, _p)

N_HEAD = 8
E = 512
L = 1024  # sequence length per batch-half
BSZ = 2
D = 64  # head dim
P = 128  # partition/chunk/pair-feature size
NCHUNK = L // P
EPS = 1e-6
N_CORES = 8
TH = 512  # token-half width for projections

# pack layouts (bf16 columns)
_P1_BIAS = 0
_P1_WQA = 8
_P1A_COLS = 520  # [bias f32-bits | wq_a]
_P1B_COLS = 512  # [wk_a]
_P2_COLS = 1024  # [wq_b | wk_b]
_P3_IDENT = 0
_P3_WV = 128
_P3_MASK = 640
_P3_COLS = 768  # [ident | wv | mask]

_CACHE = {}


def _build_bass():
    import concourse.bass as bass
    import concourse.tile as tile
    from concourse import bacc, mybir
    from contextlib import ExitStack

    f32 = mybir.dt.float32
    bf16 = mybir.dt.bfloat16
    AF = mybir.ActivationFunctionType
    D1 = D + 1

    nc = bacc.Bacc("TRN2", target_bir_lowering=False, debug=False)

    # one dram tensor per xt load: whole-tensor transfers aggregate into
    # large contiguous packets (strided slices halve the effective rate)
    xt_d = {
        (ab, th): nc.dram_tensor(f"xt{ab}{th}", [P, 2 * TH], bf16, kind="ExternalInput")
        for ab in "ab" for th in (0, 1)
    }
    p1a_d = nc.dram_tensor("p1a", [P, _P1A_COLS], bf16, kind="ExternalInput")
    p1b_d = nc.dram_tensor("p1b", [P, _P1B_COLS], bf16, kind="ExternalInput")
    p2_d = nc.dram_tensor("p2", [P, _P2_COLS], bf16, kind="ExternalInput")
    p3_d = nc.dram_tensor("p3", [P, _P3_COLS], bf16, kind="ExternalInput")
    wo_d = nc.dram_tensor("wo", [P, E], bf16, kind="ExternalInput")
    scb_d = nc.dram_tensor("scb", [P, L], bf16, kind="ExternalInput")
    out_d = nc.dram_tensor("out", [L, E], bf16, kind="ExternalOutput")

    with tile.TileContext(nc) as tc:
        with ExitStack() as ctx:
            ep = ctx.enter_context
            cpool = ep(tc.tile_pool(name="const", bufs=1))
            seqp = ep(tc.tile_pool(name="seq", bufs=1))
            bp = ep(tc.tile_pool(name="bsb", bufs=4))
            sp = ep(tc.tile_pool(name="state", bufs=4))
            atp = ep(tc.tile_pool(name="attnT", bufs=3))
            outp = ep(tc.tile_pool(name="outsb", bufs=2))
            rp = ep(tc.tile_pool(name="rcol", bufs=4))
            big_ps = ep(tc.tile_pool(name="bigps", bufs=2, space="PSUM"))
            sq_ps = ep(tc.tile_pool(name="sqps", bufs=2, space="PSUM"))
            acc_ps = ep(tc.tile_pool(name="accps", bufs=3, space="PSUM"))
            s_ps = ep(tc.tile_pool(name="sps", bufs=1, space="PSUM"))

            # ---- loads ----
            # sync ring:   p1a, e0th0, e1th0, p1b, p2, [ktA0], wo, [ktA1], st6
            # scalar ring: e2th0, e3th0, scb, p3, [ktB0], xtB_th1, [ktB1], st7
            # gpsimd:      xtA_th1, store pairs 01/23/45
            p1a_t = cpool.tile([P, _P1A_COLS], bf16, name="p1a_t")
            nc.sync.dma_start(p1a_t[:], p1a_d[:, :])
            xta = cpool.tile([P, 4 * TH], bf16, name="xta")
            xtb = cpool.tile([P, 4 * TH], bf16, name="xtb")
            nc.scalar.dma_start(xtb[:, 0 : 2 * TH], xt_d["b", 0][:, :])
            nc.sync.dma_start(xta[:, 0 : 2 * TH], xt_d["a", 0][:, :])
            p1b_t = cpool.tile([P, _P1B_COLS], bf16, name="p1b_t")
            nc.sync.dma_start(p1b_t[:], p1b_d[:, :])
            scb_sb = cpool.tile([P, L], bf16, name="scb_sb")
            nc.gpsimd.dma_start(scb_sb[:], scb_d[:, :])
            p2_t = cpool.tile([P, _P2_COLS], bf16, name="p2_t")
            nc.sync.dma_start(p2_t[:], p2_d[:, :])
            p3_t = cpool.tile([P, _P3_COLS], bf16, name="p3_t")
            nc.scalar.dma_start(p3_t[:], p3_d[:, :])
            # (xt th1 halves + wo are issued mid-stream below)

            # xslc[e][th] -> AP of the e-slice for token-half th
            def xslc(e, th):
                t = xta if e < 2 else xtb
                off = th * 2 * TH + (e % 2) * TH
                return t[:, off : off + TH]

            wt = {}
            wt["wq_a"] = [
                p1a_t[:, _P1_WQA + e * P : _P1_WQA + (e + 1) * P] for e in range(4)
            ]
            wt["wk_a"] = [p1b_t[:, e * P : (e + 1) * P] for e in range(4)]
            for wi, nm in enumerate(("wq_b", "wk_b")):
                wt[nm] = [
                    p2_t[:, wi * 512 + e * P : wi * 512 + (e + 1) * P]
                    for e in range(4)
                ]
            wt["wv"] = [
                p3_t[:, _P3_WV + e * P : _P3_WV + (e + 1) * P] for e in range(4)
            ]
            mask_t = p3_t[:, _P3_MASK : _P3_MASK + 128]
            ident_t = p3_t[:, _P3_IDENT : _P3_IDENT + 128]
            wo_t = cpool.tile([P, E], bf16, name="wo_t")
            bias4 = p1a_t[:, _P1_BIAS : _P1_BIAS + 8].bitcast(f32)
            bt = {
                nm: bias4[:, i : i + 1]
                for i, nm in enumerate(("bq_a", "bq_b", "bk_a", "bk_b"))
            }

            # ---- PE warmup on p1 (first pack to land): pstate ramp ----
            for w in range(8):
                wp = big_ps.tile([P, TH], f32, tag="big", name=f"warm{w}")
                nc.tensor.matmul(
                    wp[:], p1a_t[:, 8 : 8 + P], p1a_t[:, 8 : 8 + TH],
                    start=True, stop=True,
                )

            # ---- persistent vt pair tiles: [va | 1 | vb | 1], ones preset ----
            vt2 = [cpool.tile([P, 2 * D1], bf16, name=f"vt2_{i}") for i in range(2)]
            for i in range(2):
                nc.gpsimd.memset(vt2[i][:, D:D1], 1.0)
                nc.gpsimd.memset(vt2[i][:, D1 + D : 2 * D1], 1.0)

            q_seq = {h: seqp.tile([P, L], bf16, name=f"q_{h}") for h in "ab"}
            k_seq = {h: seqp.tile([P, L], bf16, name=f"k_{h}") for h in "ab"}
            v_seq = seqp.tile([P, L], bf16, name="v_pair")
            kt = {h: seqp.tile([P, L], bf16, name=f"kt_{h}") for h in "ab"}
            attn_all = seqp.tile([P, L], bf16, name="attn_all")

            def kt_T(h, th, eng):
                sl = slice(th * TH, (th + 1) * TH)
                dst = kt[h][:, sl].rearrange("p (c m) -> p c m", m=P)
                eng.dma_start_transpose(dst, k_seq[h][:, sl])

            def project_half(seq, wname, bname, outname, th, mul_eng):
                ps = big_ps.tile([P, TH], f32, tag="big", name=f"{outname}_ps{th}")
                for e in range(4):
                    nc.tensor.matmul(
                        ps[:], wt[wname][e], xslc(e, th),
                        start=(e == 0), stop=(e == 3),
                    )
                sl = seq[:, th * TH : (th + 1) * TH]
                if bname is None:
                    nc.scalar.copy(sl, ps[:])
                else:
                    nc.scalar.activation(sl, ps[:], AF.Relu, bias=bt[bname])
                    mul_eng.tensor_mul(sl, sl, scb_sb[:, th * TH : (th + 1) * TH])

            def project_th(th):
                eng = nc.vector if th == 0 else nc.gpsimd
                project_half(q_seq["a"], "wq_a", "bq_a", "q_a", th, eng)
                project_half(k_seq["a"], "wk_a", "bk_a", "k_a", th, eng)
                project_half(q_seq["b"], "wq_b", "bq_b", "q_b", th, eng)
                project_half(k_seq["b"], "wk_b", "bk_b", "k_b", th, eng)
                project_half(v_seq, "wv", None, "v_pair", th, None)

            # ---- attention ----
            s_pair = s_ps.tile([P, 2 * D1], f32, name="s_pair")
            state = {"prev": None}

            def attn_chunk(c):
                cs = slice(c * P, (c + 1) * P)
                vt_ps = acc_ps.tile([P, P], bf16, tag="acc", name=f"vtps{c}")
                nc.tensor.matmul(vt_ps[:], v_seq[:, cs], ident_t, is_transpose=True)
                vt = vt2[c % 2]
                nc.vector.tensor_copy(
                    vt[:].rearrange("p (b x) -> p b x", x=D1)[:, :, 0:D],
                    vt_ps[:].rearrange("p (b x) -> p b x", x=D),
                )
                vts = {"a": vt[:, 0:D1], "b": vt[:, D1 : 2 * D1]}
                bsb = {}
                for j, h in enumerate("ab"):
                    b_ps = sq_ps.tile([P, P], f32, tag="sq", name=f"bps_{h}{c}")
                    nc.tensor.matmul(
                        b_ps[:], k_seq[h][:, cs], q_seq[h][:, cs], start=True, stop=True
                    )
                    b_sb = bp.tile([P, P], bf16, tag="bsb", name=f"bsb_{h}{c}")
                    nc.vector.tensor_mul(b_sb[:], b_ps[:], mask_t)
                    bsb[h] = b_sb
                qkv = acc_ps.tile([P, 2 * D1], f32, tag="acc", name=f"qkv{c}")
                nc.tensor.matmul(
                    qkv[:, 0:D1], bsb["a"][:], vts["a"],
                    start=True, stop=False, skip_group_check=True,
                )
                nc.tensor.matmul(
                    qkv[:, D1 : 2 * D1], bsb["b"][:], vts["b"],
                    start=False, stop=(c == 0), skip_group_check=True,
                )
                if c > 0:
                    S = state["prev"]
                    nc.tensor.matmul(
                        qkv[:, 0:D1], q_seq["a"][:, cs], S[:, 0:D1],
                        start=False, stop=False, skip_group_check=True,
                    )
                    nc.tensor.matmul(
                        qkv[:, D1 : 2 * D1], q_seq["b"][:, cs], S[:, D1 : 2 * D1],
                        start=False, stop=True, skip_group_check=True,
                    )
                if c < NCHUNK - 1:
                    for j, h in enumerate("ab"):
                        nc.tensor.matmul(
                            s_pair[:, j * D1 : (j + 1) * D1],
                            kt[h][:, cs],
                            vts[h],
                            start=(c == 0 and j == 0),
                            stop=(c == NCHUNK - 2),
                            skip_group_check=True,
                        )
                    s_new = sp.tile([P, 2 * D1], bf16, tag="S", name=f"S{c}")
                    nc.scalar.copy(s_new[:], s_pair[:])
                    state["prev"] = s_new
                r_col = rp.tile([P, 4], f32, tag="r", name=f"r{c}")
                dens = qkv[:].rearrange("p (h x) -> p h x", x=D1)[:, :, D : D + 1]
                nc.vector.tensor_scalar_max(r_col[:, 0:2], dens, EPS)
                nc.vector.reciprocal(r_col[:, 2:4], r_col[:, 0:2])
                nc.vector.tensor_scalar_mul(
                    attn_all[:, c * P : c * P + D], qkv[:, 0:D], r_col[:, 2:3]
                )
                nc.vector.tensor_scalar_mul(
                    attn_all[:, c * P + D : (c + 1) * P],
                    qkv[:, D1 : D1 + D],
                    r_col[:, 3:4],
                )

            o_pair = {}

            def outproj(c):
                at_ps = acc_ps.tile([P, P], bf16, tag="acc", name=f"atps{c}")
                nc.tensor.matmul(
                    at_ps[:], attn_all[:, c * P : (c + 1) * P], ident_t,
                    is_transpose=True,
                )
                at_sb = atp.tile([P, P], bf16, tag="at", name=f"at{c}")
                nc.scalar.copy(at_sb[:], at_ps[:])
                o_ps = big_ps.tile([P, E], f32, tag="big", name=f"ops{c}")
                nc.tensor.matmul(o_ps[:], at_sb[:], wo_t[:], start=True, stop=True)
                if c >= 6:
                    # last chunks: single stores on separate queues (short tail)
                    o_sb = outp.tile([P, E], bf16, tag="osb", name=f"osb{c}")
                    if c == 6:
                        nc.scalar.copy(o_sb[:], o_ps[:])
                        nc.scalar.dma_start(out_d[c * P : (c + 1) * P, :], o_sb[:])
                    else:
                        nc.vector.tensor_copy(o_sb[:], o_ps[:])
                        nc.sync.dma_start(out_d[c * P : (c + 1) * P, :], o_sb[:])
                    return
                cp = c // 2
                if c % 2 == 0:
                    o_pair[cp] = outp.tile([P, 2 * E], bf16, tag="osb", name=f"op{cp}")
                    nc.scalar.copy(o_pair[cp][:, 0:E], o_ps[:])
                else:
                    nc.vector.tensor_copy(o_pair[cp][:, E : 2 * E], o_ps[:])
                    dst = out_d[cp * 2 * P : (cp + 1) * 2 * P, :].rearrange(
                        "(b r) e -> r b e", b=2
                    )
                    src = o_pair[cp][:].rearrange("p (b e) -> p b e", e=E)
                    eng = nc.gpsimd
                    eng.dma_start(dst, src)

            project_th(0)
            kt_T("a", 0, nc.sync)
            kt_T("b", 0, nc.scalar)
            nc.sync.dma_start(xta[:, 2 * TH : 4 * TH], xt_d["a", 1][:, :])
            nc.scalar.dma_start(xtb[:, 2 * TH : 4 * TH], xt_d["b", 1][:, :])
            nc.sync.dma_start(wo_t[:], wo_d[:, :])
            attn_chunk(0)
            attn_chunk(1)
            outproj(0)
            attn_chunk(2)
            outproj(1)
            attn_chunk(3)
            outproj(2)
            project_th(1)
            kt_T("a", 1, nc.sync)
            kt_T("b", 1, nc.scalar)
            outproj(3)
            attn_chunk(4)
            attn_chunk(5)
            outproj(4)
            attn_chunk(6)
            outproj(5)
            attn_chunk(7)
            outproj(6)
            outproj(7)

    nc.compile()
    return nc


def _get_nc():
    if "nc" not in _CACHE:
        _CACHE["nc"] = _build_bass()
    return _CACHE["nc"]


def make_in_maps(query, Wq, bq, Wk, bk, Wv, bv, Wo, bo):
    import ml_dtypes

    f32 = np.float32
    bf16 = ml_dtypes.bfloat16
    query = np.asarray(query, f32)
    x3 = query.reshape(L, BSZ, E)  # faithful torch .view reshape
    idx = (np.pi / 2) * np.arange(1, L + 1, dtype=f32) / f32(L)
    sinv = np.sin(idx).astype(f32)
    cosv = np.cos(idx).astype(f32)

    Wq, Wk, Wv, Wo = (np.asarray(w, f32) for w in (Wq, Wk, Wv, Wo))
    bq, bk, bv = (np.asarray(b, f32) for b in (bq, bk, bv))

    def wslice_dup(W, h):
        """(128, 512): [Wh.T | Wh.T] dup cols laid out as 4 e-tiles of 128."""
        w = W[D * h : D * (h + 1), :].T  # (512, 64)
        wd = np.concatenate([w, w], axis=1)  # (512, 128)
        return np.hstack([wd[e * P : (e + 1) * P, :] for e in range(4)])

    def bdup(b, h):
        bb = b[D * h : D * (h + 1)]
        return np.concatenate([bb, bb]).astype(f32)

    ident = np.eye(P, dtype=bf16)
    scb = np.empty((P, L), f32)
    scb[0:D] = sinv[None, :]
    scb[D:P] = cosv[None, :]
    mask = np.triu(np.ones((P, P), f32)).astype(bf16)

    in_maps = []
    for c in range(N_CORES):
        i, p = divmod(c, 4)
        hA, hB = 2 * p, 2 * p + 1

        biases = np.stack(
            [bdup(bq, hA), bdup(bq, hB), bdup(bk, hA), bdup(bk, hB)], axis=1
        ).astype(f32)  # (128, 4)
        bias_bits = np.ascontiguousarray(biases).view(bf16)  # (128, 8)
        p1a = np.hstack([bias_bits, wslice_dup(Wq, hA).astype(bf16)])
        p1b = wslice_dup(Wk, hA).astype(bf16)
        assert p1a.shape == (P, _P1A_COLS), p1a.shape
        assert p1b.shape == (P, _P1B_COLS), p1b.shape
        p2 = np.hstack(
            [wslice_dup(Wq, hB).astype(bf16), wslice_dup(Wk, hB).astype(bf16)]
        )
        assert p2.shape == (P, _P2_COLS), p2.shape

        wv_p = Wv[P * p : P * (p + 1), :].T  # (512, 128)
        wv_pack = np.hstack([wv_p[e * P : (e + 1) * P, :] for e in range(4)])
        p3 = np.hstack([ident, wv_pack.astype(bf16), mask])
        assert p3.shape == (P, _P3_COLS), p3.shape
        wo_pack = Wo[:, P * p : P * (p + 1)].T.astype(bf16)  # (128, 512)

        xt = np.ascontiguousarray(x3[:, i, :].T).astype(bf16)  # (512, 1024)

        def xpack(e0, e1, th):
            lo, hi = (0, TH) if th == 0 else (TH, L)
            return np.hstack(
                [xt[e0 * P : (e0 + 1) * P, lo:hi], xt[e1 * P : (e1 + 1) * P, lo:hi]]
            )

        in_maps.append(
            dict(
                xta0=np.ascontiguousarray(xpack(0, 1, 0)),
                xta1=np.ascontiguousarray(xpack(0, 1, 1)),
                xtb0=np.ascontiguousarray(xpack(2, 3, 0)),
                xtb1=np.ascontiguousarray(xpack(2, 3, 1)),
                p1a=np.ascontiguousarray(p1a),
                p1b=np.ascontiguousarray(p1b),
                p2=np.ascontiguousarray(p2),
                p3=np.ascontiguousarray(p3),
                wo=np.ascontiguousarray(wo_pack),
                scb=np.ascontiguousarray(scb.astype(bf16)),
            )
        )
    return in_maps


def assemble(partials, bo, bv, Wo):
    out_flat = np.zeros((BSZ * L, E), np.float32)
    ps = [np.asarray(p, np.float32) for p in partials]
    out_flat[0::2] = ps[0] + ps[1] + ps[2] + ps[3]
    out_flat[1::2] = ps[4] + ps[5] + ps[6] + ps[7]
    # V-bias passes through the normalized attention additively (exact up to
    # the eps clip): attn(v + bv) = attn(v) + bv, so fold bv @ Wo.T into bo.
    bo_eff = np.asarray(bo, np.float32) + np.asarray(bv, np.float32) @ np.asarray(
        Wo, np.float32
    ).T.astype(np.float32)
    out_flat += bo_eff[None, :]
    return out_flat.reshape(BSZ, L, E)


def run(inputs, trace=False):
    from concourse.bass_utils import run_bass_kernel_spmd

    in_maps = make_in_maps(**inputs)
    nc = _get_nc()
    res = run_bass_kernel_spmd(nc, in_maps, list(range(N_CORES)), trace=trace)
    partials = [r["out"] for r in res.results]
    return assemble(partials, inputs["bo"], inputs["bv"], inputs["Wo"]), res


def kernel(**inputs):
    out, _ = run(inputs, trace=False)
    return out
